# revision 11
# baseline (speedup 1.0000x reference)
"""Trainium2 Bass kernel v4 for nn_Encoder_ATTENTION (gnn_message_passing).

Math per (b, n) row r:
  d      = <e_r, g_r>,  g = wn[rid] (host-gathered unit hyperplanes)
  etr    = e - d*g                          (never materialized on device)
  h^T    = We^T eT + We^T (-dG)T + zw[b]    (fp8 DoubleRow MMs, h on partitions)
  alpha  = u . tanh(h)                      (fp8 DoubleRow MM, out [1, rows])
  coeff  = softmax_b(alpha)*mask + rw*mask  (slab layout [16 hg, 512])
  out[b] = sum_r coeff*e  +  sum_r coeff*(-dG)   (two accumulating MMs)

Layouts:
  rows r = 512*hg + rr, hg = 0..63 half-groups of 4 tiles (128 rows each).
  eT fp8 comes pre-transposed from host; (-dG)T is XBAR-transposed bf16 then
  cast fp8 on DVE. zw = Wz^T z + bias built on device, slab-staged via DRAM
  round-trip into [4, 2, 16, 512] DoubleRow lhsT layout.
"""

import sys


def _ensure_path():
    for p in ("/opt/trn_rl_repo", "/root/.axon_site/_ro/trn_rl_repo"):
        if p not in sys.path:
            sys.path.append(p)


_ensure_path()

from contextlib import ExitStack

import ml_dtypes
import numpy as np

import concourse.bacc as bacc
import concourse.bass as bass
import concourse.tile as tile
from concourse import mybir
from concourse.bass import IndirectOffsetOnAxis

B, NB, DIM = 4096, 64, 256
H = 2 * DIM
NCORES = 8
BC = B // NCORES            # 512 batch rows per core
ROWS = BC * NB              # 32768 rows per core
NT = ROWS // 128            # 256 tiles
HGT = 4                     # tiles per half-group
HGR = HGT * 128             # 512 rows per half-group
NHG = ROWS // HGR           # 64 half-groups
SLAB = 16                   # half-groups per softmax slab
NSLAB = NHG // SLAB         # 4
CNT_E = 1000
N_WR = CNT_E + 1
N_ZQ = CNT_E

f32 = mybir.dt.float32
bf16 = mybir.dt.bfloat16
fp8 = mybir.dt.float8e4
i32 = mybir.dt.int32
AF = mybir.ActivationFunctionType
OP = mybir.AluOpType
DR = mybir.MatmulPerfMode.DoubleRow

BF = ml_dtypes.bfloat16
F8 = ml_dtypes.float8_e4m3


def build_nc():
    nc = bacc.Bacc("TRN2")

    e_d = nc.dram_tensor("e", [ROWS, DIM], bf16, kind="ExternalInput")
    g_d = nc.dram_tensor("g", [ROWS, DIM], bf16, kind="ExternalInput")
    wef8_d = nc.dram_tensor("wef8", [128, 2, H], fp8, kind="ExternalInput")
    wzf8_d = nc.dram_tensor("wzf8", [128, 2, H], fp8, kind="ExternalInput")
    uf8_d = nc.dram_tensor("uf8", [128, 4, 128], fp8, kind="ExternalInput")
    oh2_d = nc.dram_tensor("oh2", [4, 2, HGR], fp8, kind="ExternalInput")
    biasb_d = nc.dram_tensor("biasb", [128, H], f32, kind="ExternalInput")
    mask32_d = nc.dram_tensor("mask32", [SLAB, NSLAB, HGR], f32, kind="ExternalInput")
    rwm32_d = nc.dram_tensor("rwm32", [SLAB, NSLAB, HGR], f32, kind="ExternalInput")
    czm_d = nc.dram_tensor("czm", [128, HGT, 8], bf16, kind="ExternalInput")
    zq_d = nc.dram_tensor("zq", [N_ZQ, DIM], bf16, kind="ExternalInput")
    qoff_d = nc.dram_tensor("qoff", [128, BC // 128], i32, kind="ExternalInput")
    zwtmp_d = nc.dram_tensor("zwtmp", [BC, H], fp8, kind="ExternalOutput")
    out_d = nc.dram_tensor("out", [BC, DIM], f32, kind="ExternalOutput")

    with tile.TileContext(nc) as tc, ExitStack() as ctx:
        const = ctx.enter_context(tc.tile_pool(name="const", bufs=1))
        epool = ctx.enter_context(tc.tile_pool(name="epool", bufs=3))
        erp = ctx.enter_context(tc.tile_pool(name="erp", bufs=21))
        gpool = ctx.enter_context(tc.tile_pool(name="gpool", bufs=2))
        etp = ctx.enter_context(tc.tile_pool(name="etp", bufs=2))
        xpool = ctx.enter_context(tc.tile_pool(name="xpool", bufs=2))
        dpool = ctx.enter_context(tc.tile_pool(name="dpool", bufs=2))
        dgp = ctx.enter_context(tc.tile_pool(name="dgp", bufs=2))
        dtp = ctx.enter_context(tc.tile_pool(name="dtp", bufs=3))
        dtf = ctx.enter_context(tc.tile_pool(name="dtf", bufs=3))
        thp = ctx.enter_context(tc.tile_pool(name="thp", bufs=2))
        ebp = ctx.enter_context(tc.tile_pool(name="ebp", bufs=2))
        alp = ctx.enter_context(tc.tile_pool(name="alp", bufs=2))
        smp = ctx.enter_context(tc.tile_pool(name="smp", bufs=2))
        ctp = ctx.enter_context(tc.tile_pool(name="ctp", bufs=2))
        czp = ctx.enter_context(tc.tile_pool(name="czp", bufs=3))
        osp = ctx.enter_context(tc.tile_pool(name="osp", bufs=3))
        zwp = ctx.enter_context(tc.tile_pool(name="zwp", bufs=1))

        hps = ctx.enter_context(tc.tile_pool(name="hps", bufs=2, space="PSUM"))
        ups = ctx.enter_context(tc.tile_pool(name="ups", bufs=2, space="PSUM"))
        tps = ctx.enter_context(tc.tile_pool(name="tps", bufs=2, space="PSUM"))

        # ---------- constants ----------
        wef8 = const.tile([128, 2, H], fp8)
        nc.sync.dma_start(out=wef8[:], in_=wef8_d[:])
        wzf8 = const.tile([128, 2, H], fp8)
        nc.sync.dma_start(out=wzf8[:], in_=wzf8_d[:])
        uf8 = const.tile([128, 4, 128], fp8)
        nc.sync.dma_start(out=uf8[:], in_=uf8_d[:])
        oh2 = const.tile([4, 2, HGR], fp8)
        nc.sync.dma_start(out=oh2[:], in_=oh2_d[:])
        biasb = const.tile([128, H], f32)
        nc.sync.dma_start(out=biasb[:], in_=biasb_d[:])
        mask32 = const.tile([SLAB, NSLAB, HGR], f32)
        nc.sync.dma_start(out=mask32[:], in_=mask32_d[:])
        rwm32 = const.tile([SLAB, NSLAB, HGR], f32)
        nc.sync.dma_start(out=rwm32[:], in_=rwm32_d[:])
        czm = const.tile([128, HGT, 8], bf16)
        nc.sync.dma_start(out=czm[:], in_=czm_d[:])
        qoffs = const.tile([128, BC // 128], i32)
        nc.sync.dma_start(out=qoffs[:], in_=qoff_d[:])

        # ---------- prologue: zw = Wz^T z + bias, fp8, staged via DRAM ----------
        z_all = const.tile([128, BC // 128, DIM], bf16)
        for j in range(BC // 128):
            nc.gpsimd.indirect_dma_start(
                out=z_all[:, j, :],
                out_offset=None,
                in_=zq_d[:],
                in_offset=IndirectOffsetOnAxis(ap=qoffs[:, j : j + 1], axis=0),
            )
        zT = const.tile([128, BC // 128, 2, 128], bf16)
        for j in range(BC // 128):
            nc.sync.dma_start_transpose(out=zT[:, j, :, :], in_=z_all[:, j, :])
        zTf8 = const.tile([128, BC // 128, 2, 128], fp8)
        nc.vector.tensor_copy(zTf8[:], zT[:])
        zw_f8 = const.tile([128, BC // 128, H], fp8)
        for j in range(BC // 128):
            zwps = hps.tile([128, 2, H], f32, tag="hps")
            nc.tensor.matmul(
                out=zwps[:, 0, :], lhsT=zTf8[:, j, :, :], rhs=wzf8[:],
                start=True, stop=True, perf_mode=DR, skip_group_check=True,
            )
            nc.vector.tensor_tensor(
                out=zw_f8[:, j, :], in0=zwps[:, 0, :], in1=biasb[:], op=OP.add
            )
        zwtmp_re = zwtmp_d[:].rearrange("(c p) o -> p c o", p=128)
        nc.sync.dma_start(out=zwtmp_re, in_=zw_f8[:])
        # slab view of zwtmp rows: b = 8*(16s + j) + (4i + p)
        zwtmp_sl = zwtmp_d[:].rearrange("(g s) o -> g s o", s=8)  # [64, 8, H]

        def load_zw_slab(s, buf):
            for p in range(4):
                for i in range(2):
                    nc.sync.dma_start(
                        out=buf[p : p + 1, i, :, :],
                        in_=zwtmp_sl[SLAB * s : SLAB * (s + 1), 4 * i + p, :],
                    )

        zw_buf0 = zwp.tile([4, 2, SLAB, H], fp8, tag="zw0")
        zw_buf1 = zwp.tile([4, 2, SLAB, H], fp8, tag="zw1")
        zw_bufs = [zw_buf0, zw_buf1]
        load_zw_slab(0, zw_bufs[0])

        # ---------- main loop ----------
        e_re = e_d[:].rearrange("(t p) d -> p t d", p=128)  # [128, NT, DIM]
        g_re = g_d[:].rearrange("(t p) d -> p t d", p=128)
        st = {}
        slabst = {}

        def proj_phase(h):
            t0 = h * HGT
            e4 = epool.tile([128, HGT, DIM], bf16, tag="e4")
            nc.sync.dma_start(out=e4[:], in_=e_re[:, t0 : t0 + HGT, :])
            G4 = gpool.tile([128, HGT, DIM], bf16, tag="G4")
            nc.sync.dma_start(out=G4[:], in_=g_re[:, t0 : t0 + HGT, :])
            X4 = xpool.tile([128, HGT, DIM], bf16, tag="X4")
            nc.vector.tensor_tensor(out=X4[:], in0=e4[:], in1=G4[:], op=OP.mult)
            d4 = dpool.tile([128, HGT + 4], f32, tag="d4")
            nc.vector.tensor_reduce(
                out=d4[:, :HGT], in_=X4[:], axis=mybir.AxisListType.X, op=OP.add
            )
            nc.vector.tensor_scalar(
                out=d4[:, 4:], in0=d4[:, :HGT], scalar1=-1.0, scalar2=None, op0=OP.mult
            )
            dG4 = dgp.tile([128, HGT, DIM], bf16, tag="dG4")
            for t in range(HGT):
                nc.vector.tensor_scalar(
                    out=dG4[:, t, :], in0=G4[:, t, :],
                    scalar1=d4[:, 4 + t : 5 + t], scalar2=None, op0=OP.mult,
                )
            etr4 = erp.tile([128, HGT, DIM], bf16, tag="etr4")
            nc.vector.tensor_tensor(out=etr4[:], in0=e4[:], in1=dG4[:], op=OP.add)
            eTT = dtp.tile([128, 2 * HGT, 128], bf16, tag="eTT")
            nc.sync.dma_start_transpose(
                out=eTT[:], in_=etr4[:].rearrange("p a b -> p (a b)")
            )
            st[h] = dict(etr4=etr4, eTT=eTT)

        def proj2_phase(h):
            d = st[h]
            eTf8 = dtf.tile([128, 2 * HGT, 128], fp8, tag="eTf8")
            nc.vector.tensor_copy(eTf8[:], d.pop("eTT")[:])
            d["eTf8"] = eTf8

        def attn_phase(h):
            s = h // SLAB
            zwb = zw_bufs[s % 2]
            sh = h % SLAB
            d = st[h]
            eTf8 = d["eTf8"]
            eTr = eTf8[:].rearrange("p (t k) r -> p k t r", k=2)
            th4 = thp.tile([128, 4, HGR], fp8, tag="th4")
            hA = hps.tile([128, 2, H], f32, tag="hps")
            hB = hps.tile([128, 2, H], f32, tag="hps")
            for hc in range(4):
                dst = hA[:, hc, :] if hc < 2 else hB[:, hc - 2, :]
                wsl = wef8[:, :, 128 * hc : 128 * (hc + 1)]
                nc.tensor.matmul(
                    out=dst, lhsT=wsl, rhs=eTr,
                    start=True, stop=False, perf_mode=DR, skip_group_check=True,
                )
                nc.tensor.matmul(
                    out=dst,
                    lhsT=zwb[:, :, sh, 128 * hc : 128 * (hc + 1)],
                    rhs=oh2[:],
                    start=False, stop=True, perf_mode=DR, skip_group_check=True,
                )
                if hc == 1:
                    nc.scalar.activation(out=th4[:, 0:2, :], in_=hA[:], func=AF.Tanh)
            nc.scalar.activation(out=th4[:, 2:4, :], in_=hB[:], func=AF.Tanh)

            up = ups.tile([128, HGR], f32, tag="ups")
            nc.tensor.matmul(
                out=up[:], lhsT=uf8[:, 0:2, :], rhs=th4[:, 0:2, :],
                start=True, stop=False, perf_mode=DR, skip_group_check=True,
            )
            nc.tensor.matmul(
                out=up[:], lhsT=uf8[:, 2:4, :], rhs=th4[:, 2:4, :],
                start=False, stop=True, perf_mode=DR, skip_group_check=True,
            )
            if sh == 0:
                Ebt = ebp.tile([SLAB, HGR], f32, tag="Eb")
                slabst[s] = dict(Eb=Ebt)
                if s + 1 < NSLAB:
                    load_zw_slab(s + 1, zw_bufs[(s + 1) % 2])
            ebt = alp.tile([1, HGR], f32, tag="ebt")
            nc.scalar.activation(out=ebt[:], in_=up[0:1, :], func=AF.Exp)
            nc.sync.dma_start(out=slabst[s]["Eb"][sh : sh + 1, :], in_=ebt[:])

        def softmax_phase(s):
            d = slabst[s]
            Eb = d["Eb"]
            Em = smp.tile([SLAB, HGR], f32, tag="Em")
            nc.vector.tensor_tensor(
                out=Em[:], in0=Eb[:], in1=mask32[:, s, :], op=OP.mult
            )
            S16 = smp.tile([SLAB, 8 + 8], f32, tag="S16")
            nc.vector.tensor_reduce(
                out=S16[:, :8],
                in_=Em[:].rearrange("j (b n) -> j b n", n=64),
                axis=mybir.AxisListType.X, op=OP.add,
            )
            nc.vector.reciprocal(S16[:, 8:], S16[:, :8])
            co = smp.tile([SLAB, HGR], f32, tag="co")
            nc.vector.tensor_tensor(
                out=co[:].rearrange("j (b n) -> j b n", n=64),
                in0=Em[:].rearrange("j (b n) -> j b n", n=64),
                in1=S16[:, 8:].to_broadcast((SLAB, 8, 64)),
                op=OP.mult,
            )
            cob = smp.tile([SLAB, HGR], bf16, tag="cob")
            nc.vector.tensor_tensor(
                out=cob[:], in0=co[:], in1=rwm32[:, s, :], op=OP.add
            )
            cT = ctp.tile([128, HGT, SLAB], bf16, tag="cT")
            for t in range(HGT):
                nc.sync.dma_start_transpose(
                    out=cT[:, t, :], in_=cob[:, 128 * t : 128 * (t + 1)]
                )
            d["cT"] = cT

        def tail_phase(h):
            s = h // SLAB
            sh = h % SLAB
            d = st.pop(h)
            etr4 = d["etr4"]
            cT = slabst[s]["cT"]
            cz = czp.tile([128, HGT, 8], bf16, tag="cz")
            nc.vector.tensor_tensor(
                out=cz[:], in0=czm[:],
                in1=cT[:, :, sh : sh + 1].to_broadcast((128, HGT, 8)),
                op=OP.mult,
            )
            tp = tps.tile([8, DIM], f32, tag="tps")
            for t in range(HGT):
                nc.tensor.matmul(
                    out=tp[:], lhsT=cz[:, t, :], rhs=etr4[:, t, :],
                    start=(t == 0), stop=(t == HGT - 1), skip_group_check=True,
                )
            outS = osp.tile([8, DIM], f32, tag="outS")
            nc.scalar.copy(outS[:], tp[:])
            nc.sync.dma_start(out=out_d[8 * h : 8 * (h + 1), :], in_=outS[:])

        TAIL_LAG = SLAB + 3
        for k in range(NHG + TAIL_LAG + 2):
            if k < NHG:
                proj_phase(k)
            if 1 <= k < NHG + 1:
                proj2_phase(k - 1)
            if 2 <= k < NHG + 2:
                attn_phase(k - 2)
                if (k - 2) % SLAB == SLAB - 1:
                    softmax_phase((k - 2) // SLAB)
            j = k - 2 - TAIL_LAG
            if 0 <= j < NHG:
                tail_phase(j)

    nc.finalize()
    return nc


_NC = None


def _get_nc():
    global _NC
    if _NC is None:
        _NC = build_nc()
    return _NC


def _prep_in_maps(inputs):
    e = np.asarray(inputs["batch_nei_e_emb"], dtype=np.float32).astype(BF)
    rid = np.asarray(inputs["batch_nei_rid"]).astype(np.int32)
    rw = np.asarray(inputs["batch_nei_rw"], dtype=np.float32)
    qr = np.asarray(inputs["batch_q_rid"]).astype(np.int32)

    w = np.asarray(inputs["w_r_weight"], dtype=np.float32)
    nrm = np.maximum(np.linalg.norm(w, axis=1, keepdims=True), 1e-12)
    wn = (w / nrm).astype(BF)  # [N_WR, DIM]
    WT = np.asarray(inputs["attn_W_w"], dtype=np.float32).T  # [in=512, out=512]
    wzf8 = np.ascontiguousarray(
        WT[:256].reshape(2, 128, H).transpose(1, 0, 2)
    ).astype(F8)
    wef8 = np.ascontiguousarray(
        WT[256:].reshape(2, 128, H).transpose(1, 0, 2)
    ).astype(F8)
    zq = np.ascontiguousarray(np.asarray(inputs["zq_weight"], dtype=np.float32)).astype(BF)
    bias = np.asarray(inputs["attn_W_b"], dtype=np.float32).reshape(1, H)
    biasb = np.ascontiguousarray(np.broadcast_to(bias, (128, H)))
    ua = np.asarray(inputs["u_a_w"], dtype=np.float32).reshape(H)
    uf8 = np.ascontiguousarray(np.broadcast_to(ua.reshape(4, 128).T[:, :, None], (128, 4, 128))).astype(F8)

    # onehot for zw-add: oh2[p, i, r] = 1 if r//64 == 4i+p
    rr = np.arange(HGR) // 64
    oh2 = np.zeros((4, 2, HGR), np.float32)
    for p in range(4):
        for i in range(2):
            oh2[p, i] = (rr == 4 * i + p)
    oh2 = oh2.astype(F8)
    # czm[p, t, j] = 1 if j == 2t + p//64
    pp = np.arange(128)[:, None, None] // 64
    tt = np.arange(HGT)[None, :, None]
    jj = np.arange(8)[None, None, :]
    czm = (jj == 2 * tt + pp).astype(BF)

    in_maps = []
    for c in range(NCORES):
        sl = slice(BC * c, BC * (c + 1))
        ec = np.ascontiguousarray(e[sl].reshape(ROWS, DIM))
        ridc = rid[sl].reshape(ROWS)
        rwc = rw[sl].reshape(ROWS)
        qc = qr[sl]
        gc = np.ascontiguousarray(wn[ridc])  # [ROWS, DIM]
        maskc = (ridc < CNT_E).astype(np.float32)
        m32 = np.ascontiguousarray(
            maskc.reshape(NSLAB, SLAB, HGR).transpose(1, 0, 2)
        )
        rwm = np.ascontiguousarray(
            (rwc * maskc).reshape(NSLAB, SLAB, HGR).transpose(1, 0, 2)
        )
        in_maps.append(
            {
                "e": ec,
                "g": gc,
                "wef8": wef8,
                "wzf8": wzf8,
                "uf8": uf8,
                "oh2": oh2,
                "biasb": biasb,
                "mask32": m32,
                "rwm32": rwm,
                "czm": czm,
                "zq": zq,
                "qoff": np.ascontiguousarray(qc.reshape(BC // 128, 128).T),
            }
        )
    return in_maps


def run_cores(inputs, trace=False, tmpdir=None):
    from concourse.bass_utils import run_bass_kernel_spmd

    nc = _get_nc()
    in_maps = _prep_in_maps(inputs)
    res = run_bass_kernel_spmd(
        nc, in_maps, core_ids=list(range(NCORES)), trace=trace, tmpdir=tmpdir
    )
    out = np.concatenate([res.results[c]["out"] for c in range(NCORES)], axis=0)
    return out, res


def kernel(**inputs):
    out, _ = run_cores(inputs, trace=False)
    return out


# revision 13
# speedup vs baseline: 1.0890x; 1.0890x over previous
"""Trainium2 Bass kernel v4 for nn_Encoder_ATTENTION (gnn_message_passing).

Math per (b, n) row r:
  d      = <e_r, g_r>,  g = wn[rid] (host-gathered unit hyperplanes)
  etr    = e - d*g                          (never materialized on device)
  h^T    = We^T eT + We^T (-dG)T + zw[b]    (fp8 DoubleRow MMs, h on partitions)
  alpha  = u . tanh(h)                      (fp8 DoubleRow MM, out [1, rows])
  coeff  = softmax_b(alpha)*mask + rw*mask  (slab layout [16 hg, 512])
  out[b] = sum_r coeff*e  +  sum_r coeff*(-dG)   (two accumulating MMs)

Layouts:
  rows r = 512*hg + rr, hg = 0..63 half-groups of 4 tiles (128 rows each).
  eT fp8 comes pre-transposed from host; (-dG)T is XBAR-transposed bf16 then
  cast fp8 on DVE. zw = Wz^T z + bias built on device, slab-staged via DRAM
  round-trip into [4, 2, 16, 512] DoubleRow lhsT layout.
"""

import sys


def _ensure_path():
    for p in ("/opt/trn_rl_repo", "/root/.axon_site/_ro/trn_rl_repo"):
        if p not in sys.path:
            sys.path.append(p)


_ensure_path()

from contextlib import ExitStack

import ml_dtypes
import numpy as np

import concourse.bacc as bacc
import concourse.bass as bass
import concourse.tile as tile
from concourse import mybir
from concourse.bass import IndirectOffsetOnAxis

B, NB, DIM = 4096, 64, 256
H = 2 * DIM
NCORES = 8
BC = B // NCORES            # 512 batch rows per core
ROWS = BC * NB              # 32768 rows per core
NT = ROWS // 128            # 256 tiles
HGT = 4                     # tiles per half-group
HGR = HGT * 128             # 512 rows per half-group
NHG = ROWS // HGR           # 64 half-groups
SLAB = 16                   # half-groups per softmax slab
NSLAB = NHG // SLAB         # 4
CNT_E = 1000
N_WR = CNT_E + 1
N_ZQ = CNT_E

f32 = mybir.dt.float32
bf16 = mybir.dt.bfloat16
fp8 = mybir.dt.float8e4
i32 = mybir.dt.int32
AF = mybir.ActivationFunctionType
OP = mybir.AluOpType
DR = mybir.MatmulPerfMode.DoubleRow

BF = ml_dtypes.bfloat16
F8 = ml_dtypes.float8_e4m3


def build_nc():
    nc = bacc.Bacc("TRN2")

    e_d = nc.dram_tensor("e", [ROWS, DIM], bf16, kind="ExternalInput")
    g_d = nc.dram_tensor("g", [ROWS, DIM], bf16, kind="ExternalInput")
    wef8_d = nc.dram_tensor("wef8", [128, 2, H], fp8, kind="ExternalInput")
    wzf8_d = nc.dram_tensor("wzf8", [128, 2, H], fp8, kind="ExternalInput")
    uf8_d = nc.dram_tensor("uf8", [128, 4, 128], fp8, kind="ExternalInput")
    oh2_d = nc.dram_tensor("oh2", [4, 2, HGR], fp8, kind="ExternalInput")
    biasb_d = nc.dram_tensor("biasb", [128, H], f32, kind="ExternalInput")
    mask32_d = nc.dram_tensor("mask32", [SLAB, NSLAB, HGR], f32, kind="ExternalInput")
    rwm32_d = nc.dram_tensor("rwm32", [SLAB, NSLAB, HGR], f32, kind="ExternalInput")
    czm_d = nc.dram_tensor("czm", [128, HGT, 8], bf16, kind="ExternalInput")
    zq_d = nc.dram_tensor("zq", [N_ZQ, DIM], bf16, kind="ExternalInput")
    qoff_d = nc.dram_tensor("qoff", [128, BC // 128], i32, kind="ExternalInput")
    zwtmp_d = nc.dram_tensor("zwtmp", [BC, H], fp8, kind="ExternalOutput")
    out_d = nc.dram_tensor("out", [BC, DIM], f32, kind="ExternalOutput")

    with tile.TileContext(nc) as tc, ExitStack() as ctx:
        const = ctx.enter_context(tc.tile_pool(name="const", bufs=1))
        epool = ctx.enter_context(tc.tile_pool(name="epool", bufs=3))
        erp = ctx.enter_context(tc.tile_pool(name="erp", bufs=23))
        gpool = ctx.enter_context(tc.tile_pool(name="gpool", bufs=2))
        etp = ctx.enter_context(tc.tile_pool(name="etp", bufs=2))
        xpool = ctx.enter_context(tc.tile_pool(name="xpool", bufs=2))
        dpool = ctx.enter_context(tc.tile_pool(name="dpool", bufs=2))
        dgp = ctx.enter_context(tc.tile_pool(name="dgp", bufs=2))
        dtp = ctx.enter_context(tc.tile_pool(name="dtp", bufs=3))
        dtf = ctx.enter_context(tc.tile_pool(name="dtf", bufs=4))
        thp = ctx.enter_context(tc.tile_pool(name="thp", bufs=2))
        ebp = ctx.enter_context(tc.tile_pool(name="ebp", bufs=2))
        alp = ctx.enter_context(tc.tile_pool(name="alp", bufs=2))
        smp = ctx.enter_context(tc.tile_pool(name="smp", bufs=2))
        ctp = ctx.enter_context(tc.tile_pool(name="ctp", bufs=2))
        czp = ctx.enter_context(tc.tile_pool(name="czp", bufs=3))
        osp = ctx.enter_context(tc.tile_pool(name="osp", bufs=3))
        zwp = ctx.enter_context(tc.tile_pool(name="zwp", bufs=1))

        hps = ctx.enter_context(tc.tile_pool(name="hps", bufs=2, space="PSUM"))
        ups = ctx.enter_context(tc.tile_pool(name="ups", bufs=2, space="PSUM"))
        tps = ctx.enter_context(tc.tile_pool(name="tps", bufs=2, space="PSUM"))

        # ---------- constants ----------
        wef8 = const.tile([128, 2, H], fp8)
        nc.sync.dma_start(out=wef8[:], in_=wef8_d[:])
        wzf8 = const.tile([128, 2, H], fp8)
        nc.sync.dma_start(out=wzf8[:], in_=wzf8_d[:])
        uf8 = const.tile([128, 4, 128], fp8)
        nc.sync.dma_start(out=uf8[:], in_=uf8_d[:])
        oh2 = const.tile([4, 2, HGR], fp8)
        nc.sync.dma_start(out=oh2[:], in_=oh2_d[:])
        biasb = const.tile([128, H], f32)
        nc.sync.dma_start(out=biasb[:], in_=biasb_d[:])
        mask32 = const.tile([SLAB, NSLAB, HGR], f32)
        nc.sync.dma_start(out=mask32[:], in_=mask32_d[:])
        rwm32 = const.tile([SLAB, NSLAB, HGR], f32)
        nc.sync.dma_start(out=rwm32[:], in_=rwm32_d[:])
        czm = const.tile([128, HGT, 8], bf16)
        nc.sync.dma_start(out=czm[:], in_=czm_d[:])
        qoffs = const.tile([128, BC // 128], i32)
        nc.sync.dma_start(out=qoffs[:], in_=qoff_d[:])

        # ---------- prologue: zw = Wz^T z + bias, fp8, staged via DRAM ----------
        z_all = const.tile([128, BC // 128, DIM], bf16)
        for j in range(BC // 128):
            nc.gpsimd.indirect_dma_start(
                out=z_all[:, j, :],
                out_offset=None,
                in_=zq_d[:],
                in_offset=IndirectOffsetOnAxis(ap=qoffs[:, j : j + 1], axis=0),
            )
        zT = const.tile([128, BC // 128, 2, 128], bf16)
        for j in range(BC // 128):
            nc.sync.dma_start_transpose(out=zT[:, j, :, :], in_=z_all[:, j, :])
        zTf8 = const.tile([128, BC // 128, 2, 128], fp8)
        nc.vector.tensor_copy(zTf8[:], zT[:])
        zw_f8 = const.tile([128, BC // 128, H], fp8)
        for j in range(BC // 128):
            zwps = hps.tile([128, 2, H], f32, tag="hps")
            nc.tensor.matmul(
                out=zwps[:, 0, :], lhsT=zTf8[:, j, :, :], rhs=wzf8[:],
                start=True, stop=True, perf_mode=DR, skip_group_check=True,
            )
            nc.vector.tensor_tensor(
                out=zw_f8[:, j, :], in0=zwps[:, 0, :], in1=biasb[:], op=OP.add
            )
        zwtmp_re = zwtmp_d[:].rearrange("(c p) o -> p c o", p=128)
        nc.sync.dma_start(out=zwtmp_re, in_=zw_f8[:])
        # slab view of zwtmp rows: b = 8*(16s + j) + (4i + p)
        zwtmp_sl = zwtmp_d[:].rearrange("(g s) o -> g s o", s=8)  # [64, 8, H]

        def load_zw_slab(s, buf):
            for p in range(4):
                for i in range(2):
                    nc.sync.dma_start(
                        out=buf[p : p + 1, i, :, :],
                        in_=zwtmp_sl[SLAB * s : SLAB * (s + 1), 4 * i + p, :],
                    )

        zw_buf0 = zwp.tile([4, 2, SLAB, H], fp8, tag="zw0")
        zw_buf1 = zwp.tile([4, 2, SLAB, H], fp8, tag="zw1")
        zw_bufs = [zw_buf0, zw_buf1]
        load_zw_slab(0, zw_bufs[0])

        # ---------- main loop ----------
        e_re = e_d[:].rearrange("(t p) d -> p t d", p=128)  # [128, NT, DIM]
        g_re = g_d[:].rearrange("(t p) d -> p t d", p=128)
        st = {}
        slabst = {}

        def proj_phase(h):
            t0 = h * HGT
            e4 = epool.tile([128, HGT, DIM], bf16, tag="e4")
            nc.sync.dma_start(out=e4[:], in_=e_re[:, t0 : t0 + HGT, :])
            G4 = gpool.tile([128, HGT, DIM], bf16, tag="G4")
            nc.sync.dma_start(out=G4[:], in_=g_re[:, t0 : t0 + HGT, :])
            X4 = xpool.tile([128, HGT, DIM], bf16, tag="X4")
            nc.vector.tensor_tensor(out=X4[:], in0=e4[:], in1=G4[:], op=OP.mult)
            Xf = xpool.tile([128, HGT, DIM // 2], bf16, tag="Xf")
            nc.vector.tensor_tensor(
                out=Xf[:], in0=X4[:, :, 0 : DIM // 2], in1=X4[:, :, DIM // 2 : DIM],
                op=OP.add,
            )
            d4 = dpool.tile([128, HGT + 4], f32, tag="d4")
            nc.vector.tensor_reduce(
                out=d4[:, :HGT], in_=Xf[:], axis=mybir.AxisListType.X, op=OP.add
            )
            nc.vector.tensor_scalar(
                out=d4[:, 4:], in0=d4[:, :HGT], scalar1=-1.0, scalar2=None, op0=OP.mult
            )
            dG4 = dgp.tile([128, HGT, DIM], bf16, tag="dG4")
            for t in range(HGT):
                nc.vector.tensor_scalar(
                    out=dG4[:, t, :], in0=G4[:, t, :],
                    scalar1=d4[:, 4 + t : 5 + t], scalar2=None, op0=OP.mult,
                )
            etr4 = erp.tile([128, HGT, DIM], bf16, tag="etr4")
            nc.vector.tensor_tensor(out=etr4[:], in0=e4[:], in1=dG4[:], op=OP.add)
            eTT = dtp.tile([128, 2 * HGT, 128], bf16, tag="eTT")
            nc.sync.dma_start_transpose(
                out=eTT[:], in_=etr4[:].rearrange("p a b -> p (a b)")
            )
            st[h] = dict(etr4=etr4, eTT=eTT)

        def proj2_phase(h):
            d = st[h]
            eTf8 = dtf.tile([128, 2 * HGT, 128], fp8, tag="eTf8")
            nc.vector.tensor_copy(eTf8[:], d.pop("eTT")[:])
            d["eTf8"] = eTf8

        def attn_phase(h):
            s = h // SLAB
            zwb = zw_bufs[s % 2]
            sh = h % SLAB
            d = st[h]
            eTf8 = d["eTf8"]
            eTr = eTf8[:].rearrange("p (t k) r -> p k t r", k=2)
            th4 = thp.tile([128, 4, HGR], fp8, tag="th4")
            hA = hps.tile([128, 2, H], f32, tag="hps")
            hB = hps.tile([128, 2, H], f32, tag="hps")
            for hc in range(4):
                dst = hA[:, hc, :] if hc < 2 else hB[:, hc - 2, :]
                wsl = wef8[:, :, 128 * hc : 128 * (hc + 1)]
                nc.tensor.matmul(
                    out=dst, lhsT=wsl, rhs=eTr,
                    start=True, stop=False, perf_mode=DR, skip_group_check=True,
                )
                nc.tensor.matmul(
                    out=dst,
                    lhsT=zwb[:, :, sh, 128 * hc : 128 * (hc + 1)],
                    rhs=oh2[:],
                    start=False, stop=True, perf_mode=DR, skip_group_check=True,
                )
                if hc == 1:
                    nc.scalar.activation(out=th4[:, 0:2, :], in_=hA[:], func=AF.Tanh)
            nc.scalar.activation(out=th4[:, 2:4, :], in_=hB[:], func=AF.Tanh)

            up = ups.tile([128, HGR], f32, tag="ups")
            nc.tensor.matmul(
                out=up[:], lhsT=uf8[:, 0:2, :], rhs=th4[:, 0:2, :],
                start=True, stop=False, perf_mode=DR, skip_group_check=True,
            )
            d["up"] = up
            d["th4"] = th4

        def u2_phase(h):
            s = h // SLAB
            sh = h % SLAB
            d = st[h]
            up = d.pop("up")
            th4 = d.pop("th4")
            nc.tensor.matmul(
                out=up[:], lhsT=uf8[:, 2:4, :], rhs=th4[:, 2:4, :],
                start=False, stop=True, perf_mode=DR, skip_group_check=True,
            )
            if sh == 0:
                Ebt = ebp.tile([SLAB, HGR], f32, tag="Eb")
                slabst[s] = dict(Eb=Ebt)
                if s + 1 < NSLAB:
                    load_zw_slab(s + 1, zw_bufs[(s + 1) % 2])
            ebt = alp.tile([1, HGR], f32, tag="ebt")
            nc.scalar.activation(out=ebt[:], in_=up[0:1, :], func=AF.Exp)
            nc.sync.dma_start(out=slabst[s]["Eb"][sh : sh + 1, :], in_=ebt[:])

        def softmax_phase(s):
            d = slabst[s]
            Eb = d["Eb"]
            Em = smp.tile([SLAB, HGR], f32, tag="Em")
            nc.vector.tensor_tensor(
                out=Em[:], in0=Eb[:], in1=mask32[:, s, :], op=OP.mult
            )
            S16 = smp.tile([SLAB, 8 + 8], f32, tag="S16")
            nc.vector.tensor_reduce(
                out=S16[:, :8],
                in_=Em[:].rearrange("j (b n) -> j b n", n=64),
                axis=mybir.AxisListType.X, op=OP.add,
            )
            nc.vector.reciprocal(S16[:, 8:], S16[:, :8])
            co = smp.tile([SLAB, HGR], f32, tag="co")
            nc.vector.tensor_tensor(
                out=co[:].rearrange("j (b n) -> j b n", n=64),
                in0=Em[:].rearrange("j (b n) -> j b n", n=64),
                in1=S16[:, 8:].to_broadcast((SLAB, 8, 64)),
                op=OP.mult,
            )
            cob = smp.tile([SLAB, HGR], bf16, tag="cob")
            nc.vector.tensor_tensor(
                out=cob[:], in0=co[:], in1=rwm32[:, s, :], op=OP.add
            )
            cT = ctp.tile([128, HGT, SLAB], bf16, tag="cT")
            for t in range(HGT):
                nc.sync.dma_start_transpose(
                    out=cT[:, t, :], in_=cob[:, 128 * t : 128 * (t + 1)]
                )
            d["cT"] = cT

        def tail_phase(h):
            s = h // SLAB
            sh = h % SLAB
            d = st.pop(h)
            etr4 = d["etr4"]
            cT = slabst[s]["cT"]
            cz = czp.tile([128, HGT, 8], bf16, tag="cz")
            nc.vector.tensor_tensor(
                out=cz[:], in0=czm[:],
                in1=cT[:, :, sh : sh + 1].to_broadcast((128, HGT, 8)),
                op=OP.mult,
            )
            tp = tps.tile([8, DIM], f32, tag="tps")
            for t in range(HGT):
                nc.tensor.matmul(
                    out=tp[:], lhsT=cz[:, t, :], rhs=etr4[:, t, :],
                    start=(t == 0), stop=(t == HGT - 1), skip_group_check=True,
                )
            outS = osp.tile([8, DIM], f32, tag="outS")
            nc.scalar.copy(outS[:], tp[:])
            nc.sync.dma_start(out=out_d[8 * h : 8 * (h + 1), :], in_=outS[:])

        TAIL_LAG = SLAB + 3
        for k in range(NHG + TAIL_LAG + 4):
            if k < NHG:
                proj_phase(k)
            if 1 <= k < NHG + 1:
                proj2_phase(k - 1)
            if 3 <= k < NHG + 3:
                attn_phase(k - 3)
            j = k - 3 - TAIL_LAG
            if 0 <= j < NHG:
                tail_phase(j)
            if 3 <= k < NHG + 3:
                u2_phase(k - 3)
                if (k - 3) % SLAB == SLAB - 1:
                    softmax_phase((k - 3) // SLAB)

    nc.finalize()
    return nc


_NC = None


def _get_nc():
    global _NC
    if _NC is None:
        _NC = build_nc()
    return _NC


def _prep_in_maps(inputs):
    e = np.asarray(inputs["batch_nei_e_emb"], dtype=np.float32).astype(BF)
    rid = np.asarray(inputs["batch_nei_rid"]).astype(np.int32)
    rw = np.asarray(inputs["batch_nei_rw"], dtype=np.float32)
    qr = np.asarray(inputs["batch_q_rid"]).astype(np.int32)

    w = np.asarray(inputs["w_r_weight"], dtype=np.float32)
    nrm = np.maximum(np.linalg.norm(w, axis=1, keepdims=True), 1e-12)
    wn = (w / nrm).astype(BF)  # [N_WR, DIM]
    WT = np.asarray(inputs["attn_W_w"], dtype=np.float32).T  # [in=512, out=512]
    wzf8 = np.ascontiguousarray(
        WT[:256].reshape(2, 128, H).transpose(1, 0, 2)
    ).astype(F8)
    wef8 = np.ascontiguousarray(
        WT[256:].reshape(2, 128, H).transpose(1, 0, 2)
    ).astype(F8)
    zq = np.ascontiguousarray(np.asarray(inputs["zq_weight"], dtype=np.float32)).astype(BF)
    bias = np.asarray(inputs["attn_W_b"], dtype=np.float32).reshape(1, H)
    biasb = np.ascontiguousarray(np.broadcast_to(bias, (128, H)))
    ua = np.asarray(inputs["u_a_w"], dtype=np.float32).reshape(H)
    uf8 = np.ascontiguousarray(np.broadcast_to(ua.reshape(4, 128).T[:, :, None], (128, 4, 128))).astype(F8)

    # onehot for zw-add: oh2[p, i, r] = 1 if r//64 == 4i+p
    rr = np.arange(HGR) // 64
    oh2 = np.zeros((4, 2, HGR), np.float32)
    for p in range(4):
        for i in range(2):
            oh2[p, i] = (rr == 4 * i + p)
    oh2 = oh2.astype(F8)
    # czm[p, t, j] = 1 if j == 2t + p//64
    pp = np.arange(128)[:, None, None] // 64
    tt = np.arange(HGT)[None, :, None]
    jj = np.arange(8)[None, None, :]
    czm = (jj == 2 * tt + pp).astype(BF)

    in_maps = []
    for c in range(NCORES):
        sl = slice(BC * c, BC * (c + 1))
        ec = np.ascontiguousarray(e[sl].reshape(ROWS, DIM))
        ridc = rid[sl].reshape(ROWS)
        rwc = rw[sl].reshape(ROWS)
        qc = qr[sl]
        gc = np.ascontiguousarray(wn[ridc])  # [ROWS, DIM]
        maskc = (ridc < CNT_E).astype(np.float32)
        m32 = np.ascontiguousarray(
            maskc.reshape(NSLAB, SLAB, HGR).transpose(1, 0, 2)
        )
        rwm = np.ascontiguousarray(
            (rwc * maskc).reshape(NSLAB, SLAB, HGR).transpose(1, 0, 2)
        )
        in_maps.append(
            {
                "e": ec,
                "g": gc,
                "wef8": wef8,
                "wzf8": wzf8,
                "uf8": uf8,
                "oh2": oh2,
                "biasb": biasb,
                "mask32": m32,
                "rwm32": rwm,
                "czm": czm,
                "zq": zq,
                "qoff": np.ascontiguousarray(qc.reshape(BC // 128, 128).T),
            }
        )
    return in_maps


def run_cores(inputs, trace=False, tmpdir=None):
    from concourse.bass_utils import run_bass_kernel_spmd

    nc = _get_nc()
    in_maps = _prep_in_maps(inputs)
    res = run_bass_kernel_spmd(
        nc, in_maps, core_ids=list(range(NCORES)), trace=trace, tmpdir=tmpdir
    )
    out = np.concatenate([res.results[c]["out"] for c in range(NCORES)], axis=0)
    return out, res


def kernel(**inputs):
    out, _ = run_cores(inputs, trace=False)
    return out


# revision 14
# speedup vs baseline: 1.1046x; 1.0143x over previous
"""Trainium2 Bass kernel v4 for nn_Encoder_ATTENTION (gnn_message_passing).

Math per (b, n) row r:
  d      = <e_r, g_r>,  g = wn[rid] (host-gathered unit hyperplanes)
  etr    = e - d*g                          (never materialized on device)
  h^T    = We^T eT + We^T (-dG)T + zw[b]    (fp8 DoubleRow MMs, h on partitions)
  alpha  = u . tanh(h)                      (fp8 DoubleRow MM, out [1, rows])
  coeff  = softmax_b(alpha)*mask + rw*mask  (slab layout [16 hg, 512])
  out[b] = sum_r coeff*e  +  sum_r coeff*(-dG)   (two accumulating MMs)

Layouts:
  rows r = 512*hg + rr, hg = 0..63 half-groups of 4 tiles (128 rows each).
  eT fp8 comes pre-transposed from host; (-dG)T is XBAR-transposed bf16 then
  cast fp8 on DVE. zw = Wz^T z + bias built on device, slab-staged via DRAM
  round-trip into [4, 2, 16, 512] DoubleRow lhsT layout.
"""

import sys


def _ensure_path():
    for p in ("/opt/trn_rl_repo", "/root/.axon_site/_ro/trn_rl_repo"):
        if p not in sys.path:
            sys.path.append(p)


_ensure_path()

from contextlib import ExitStack

import ml_dtypes
import numpy as np

import concourse.bacc as bacc
import concourse.bass as bass
import concourse.tile as tile
from concourse import mybir
from concourse.bass import IndirectOffsetOnAxis

B, NB, DIM = 4096, 64, 256
H = 2 * DIM
NCORES = 8
BC = B // NCORES            # 512 batch rows per core
ROWS = BC * NB              # 32768 rows per core
NT = ROWS // 128            # 256 tiles
HGT = 4                     # tiles per half-group
HGR = HGT * 128             # 512 rows per half-group
NHG = ROWS // HGR           # 64 half-groups
SLAB = 16                   # half-groups per softmax slab
NSLAB = NHG // SLAB         # 4
CNT_E = 1000
N_WR = CNT_E + 1
N_ZQ = CNT_E

f32 = mybir.dt.float32
bf16 = mybir.dt.bfloat16
fp8 = mybir.dt.float8e4
i32 = mybir.dt.int32
AF = mybir.ActivationFunctionType
OP = mybir.AluOpType
DR = mybir.MatmulPerfMode.DoubleRow

BF = ml_dtypes.bfloat16
F8 = ml_dtypes.float8_e4m3


def build_nc():
    nc = bacc.Bacc("TRN2")

    e_d = nc.dram_tensor("e", [ROWS, DIM], bf16, kind="ExternalInput")
    g_d = nc.dram_tensor("g", [ROWS, DIM], bf16, kind="ExternalInput")
    wef8_d = nc.dram_tensor("wef8", [128, 2, H], fp8, kind="ExternalInput")
    wzf8_d = nc.dram_tensor("wzf8", [128, 2, H], fp8, kind="ExternalInput")
    uf8_d = nc.dram_tensor("uf8", [128, 4, 128], fp8, kind="ExternalInput")
    oh2_d = nc.dram_tensor("oh2", [4, 2, HGR], fp8, kind="ExternalInput")
    biasb_d = nc.dram_tensor("biasb", [128, H], f32, kind="ExternalInput")
    mask32_d = nc.dram_tensor("mask32", [SLAB, NSLAB, HGR], f32, kind="ExternalInput")
    rwm32_d = nc.dram_tensor("rwm32", [SLAB, NSLAB, HGR], f32, kind="ExternalInput")
    czm_d = nc.dram_tensor("czm", [128, HGT, 8], bf16, kind="ExternalInput")
    zq_d = nc.dram_tensor("zq", [N_ZQ, DIM], bf16, kind="ExternalInput")
    qoff_d = nc.dram_tensor("qoff", [128, BC // 128], i32, kind="ExternalInput")
    zwtmp_d = nc.dram_tensor("zwtmp", [BC, H], fp8, kind="ExternalOutput")
    out_d = nc.dram_tensor("out", [BC, DIM], f32, kind="ExternalOutput")

    with tile.TileContext(nc) as tc, ExitStack() as ctx:
        const = ctx.enter_context(tc.tile_pool(name="const", bufs=1))
        epool = ctx.enter_context(tc.tile_pool(name="epool", bufs=4))
        erp = ctx.enter_context(tc.tile_pool(name="erp", bufs=23))
        gpool = ctx.enter_context(tc.tile_pool(name="gpool", bufs=3))
        etp = ctx.enter_context(tc.tile_pool(name="etp", bufs=2))
        xpool = ctx.enter_context(tc.tile_pool(name="xpool", bufs=3))
        dpool = ctx.enter_context(tc.tile_pool(name="dpool", bufs=2))
        dgp = ctx.enter_context(tc.tile_pool(name="dgp", bufs=2))
        dtp = ctx.enter_context(tc.tile_pool(name="dtp", bufs=3))
        dtf = ctx.enter_context(tc.tile_pool(name="dtf", bufs=4))
        thp = ctx.enter_context(tc.tile_pool(name="thp", bufs=3))
        ebp = ctx.enter_context(tc.tile_pool(name="ebp", bufs=2))
        alp = ctx.enter_context(tc.tile_pool(name="alp", bufs=3))
        smp = ctx.enter_context(tc.tile_pool(name="smp", bufs=2))
        ctp = ctx.enter_context(tc.tile_pool(name="ctp", bufs=2))
        czp = ctx.enter_context(tc.tile_pool(name="czp", bufs=3))
        osp = ctx.enter_context(tc.tile_pool(name="osp", bufs=3))
        zwp = ctx.enter_context(tc.tile_pool(name="zwp", bufs=1))

        hps = ctx.enter_context(tc.tile_pool(name="hps", bufs=2, space="PSUM"))
        ups = ctx.enter_context(tc.tile_pool(name="ups", bufs=2, space="PSUM"))
        tps = ctx.enter_context(tc.tile_pool(name="tps", bufs=2, space="PSUM"))

        # ---------- constants ----------
        wef8 = const.tile([128, 2, H], fp8)
        nc.sync.dma_start(out=wef8[:], in_=wef8_d[:])
        wzf8 = const.tile([128, 2, H], fp8)
        nc.sync.dma_start(out=wzf8[:], in_=wzf8_d[:])
        uf8 = const.tile([128, 4, 128], fp8)
        nc.sync.dma_start(out=uf8[:], in_=uf8_d[:])
        oh2 = const.tile([4, 2, HGR], fp8)
        nc.sync.dma_start(out=oh2[:], in_=oh2_d[:])
        biasb = const.tile([128, H], f32)
        nc.sync.dma_start(out=biasb[:], in_=biasb_d[:])
        mask32 = const.tile([SLAB, NSLAB, HGR], f32)
        nc.sync.dma_start(out=mask32[:], in_=mask32_d[:])
        rwm32 = const.tile([SLAB, NSLAB, HGR], f32)
        nc.sync.dma_start(out=rwm32[:], in_=rwm32_d[:])
        czm = const.tile([128, HGT, 8], bf16)
        nc.sync.dma_start(out=czm[:], in_=czm_d[:])
        qoffs = const.tile([128, BC // 128], i32)
        nc.sync.dma_start(out=qoffs[:], in_=qoff_d[:])

        # ---------- prologue: zw = Wz^T z + bias, fp8, staged via DRAM ----------
        z_all = const.tile([128, BC // 128, DIM], bf16)
        for j in range(BC // 128):
            nc.gpsimd.indirect_dma_start(
                out=z_all[:, j, :],
                out_offset=None,
                in_=zq_d[:],
                in_offset=IndirectOffsetOnAxis(ap=qoffs[:, j : j + 1], axis=0),
            )
        zT = const.tile([128, BC // 128, 2, 128], bf16)
        for j in range(BC // 128):
            nc.sync.dma_start_transpose(out=zT[:, j, :, :], in_=z_all[:, j, :])
        zTf8 = const.tile([128, BC // 128, 2, 128], fp8)
        nc.vector.tensor_copy(zTf8[:], zT[:])
        zw_f8 = const.tile([128, BC // 128, H], fp8)
        for j in range(BC // 128):
            zwps = hps.tile([128, 2, H], f32, tag="hps")
            nc.tensor.matmul(
                out=zwps[:, 0, :], lhsT=zTf8[:, j, :, :], rhs=wzf8[:],
                start=True, stop=True, perf_mode=DR, skip_group_check=True,
            )
            nc.vector.tensor_tensor(
                out=zw_f8[:, j, :], in0=zwps[:, 0, :], in1=biasb[:], op=OP.add
            )
        zwtmp_re = zwtmp_d[:].rearrange("(c p) o -> p c o", p=128)
        nc.sync.dma_start(out=zwtmp_re, in_=zw_f8[:])
        # slab view of zwtmp rows: b = 8*(16s + j) + (4i + p)
        zwtmp_sl = zwtmp_d[:].rearrange("(g s) o -> g s o", s=8)  # [64, 8, H]

        def load_zw_slab(s, buf):
            for p in range(4):
                for i in range(2):
                    nc.sync.dma_start(
                        out=buf[p : p + 1, i, :, :],
                        in_=zwtmp_sl[SLAB * s : SLAB * (s + 1), 4 * i + p, :],
                    )

        zw_buf0 = zwp.tile([4, 2, SLAB, H], fp8, tag="zw0")
        zw_buf1 = zwp.tile([4, 2, SLAB, H], fp8, tag="zw1")
        zw_bufs = [zw_buf0, zw_buf1]
        load_zw_slab(0, zw_bufs[0])

        # ---------- main loop ----------
        e_re = e_d[:].rearrange("(t p) d -> p t d", p=128)  # [128, NT, DIM]
        g_re = g_d[:].rearrange("(t p) d -> p t d", p=128)
        st = {}
        slabst = {}

        def proj_phase(h):
            t0 = h * HGT
            e4 = epool.tile([128, HGT, DIM], bf16, tag="e4")
            nc.sync.dma_start(out=e4[:], in_=e_re[:, t0 : t0 + HGT, :])
            G4 = gpool.tile([128, HGT, DIM], bf16, tag="G4")
            nc.sync.dma_start(out=G4[:], in_=g_re[:, t0 : t0 + HGT, :])
            X4 = xpool.tile([128, HGT, DIM], bf16, tag="X4")
            nc.vector.tensor_tensor(out=X4[:], in0=e4[:], in1=G4[:], op=OP.mult)
            Xf = xpool.tile([128, HGT, DIM // 2], bf16, tag="Xf")
            nc.vector.tensor_tensor(
                out=Xf[:], in0=X4[:, :, 0 : DIM // 2], in1=X4[:, :, DIM // 2 : DIM],
                op=OP.add,
            )
            d4 = dpool.tile([128, HGT + 4], f32, tag="d4")
            nc.vector.tensor_reduce(
                out=d4[:, :HGT], in_=Xf[:], axis=mybir.AxisListType.X, op=OP.add
            )
            nc.vector.tensor_scalar(
                out=d4[:, 4:], in0=d4[:, :HGT], scalar1=-1.0, scalar2=None, op0=OP.mult
            )
            dG4 = dgp.tile([128, HGT, DIM], bf16, tag="dG4")
            for t in range(HGT):
                nc.vector.tensor_scalar(
                    out=dG4[:, t, :], in0=G4[:, t, :],
                    scalar1=d4[:, 4 + t : 5 + t], scalar2=None, op0=OP.mult,
                )
            etr4 = erp.tile([128, HGT, DIM], bf16, tag="etr4")
            nc.vector.tensor_tensor(out=etr4[:], in0=e4[:], in1=dG4[:], op=OP.add)
            eTT = dtp.tile([128, 2 * HGT, 128], bf16, tag="eTT")
            nc.sync.dma_start_transpose(
                out=eTT[:], in_=etr4[:].rearrange("p a b -> p (a b)")
            )
            st[h] = dict(etr4=etr4, eTT=eTT)

        def proj2_phase(h):
            d = st[h]
            eTf8 = dtf.tile([128, 2 * HGT, 128], fp8, tag="eTf8")
            nc.vector.tensor_copy(eTf8[:], d.pop("eTT")[:])
            d["eTf8"] = eTf8

        def attn_phase(h):
            s = h // SLAB
            zwb = zw_bufs[s % 2]
            sh = h % SLAB
            d = st[h]
            eTf8 = d["eTf8"]
            eTr = eTf8[:].rearrange("p (t k) r -> p k t r", k=2)
            th4 = thp.tile([128, 4, HGR], fp8, tag="th4")
            hA = hps.tile([128, 2, H], f32, tag="hps")
            hB = hps.tile([128, 2, H], f32, tag="hps")
            for hc in range(4):
                dst = hA[:, hc, :] if hc < 2 else hB[:, hc - 2, :]
                wsl = wef8[:, :, 128 * hc : 128 * (hc + 1)]
                nc.tensor.matmul(
                    out=dst,
                    lhsT=zwb[:, :, sh, 128 * hc : 128 * (hc + 1)],
                    rhs=oh2[:],
                    start=True, stop=False, perf_mode=DR, skip_group_check=True,
                )
                nc.tensor.matmul(
                    out=dst, lhsT=wsl, rhs=eTr,
                    start=False, stop=True, perf_mode=DR, skip_group_check=True,
                )
                if hc == 1:
                    nc.scalar.activation(out=th4[:, 0:2, :], in_=hA[:], func=AF.Tanh)
            nc.scalar.activation(out=th4[:, 2:4, :], in_=hB[:], func=AF.Tanh)

            up = ups.tile([128, HGR], f32, tag="ups")
            nc.tensor.matmul(
                out=up[:], lhsT=uf8[:, 0:2, :], rhs=th4[:, 0:2, :],
                start=True, stop=False, perf_mode=DR, skip_group_check=True,
            )
            d["up"] = up
            d["th4"] = th4

        def u2_phase(h):
            s = h // SLAB
            sh = h % SLAB
            d = st[h]
            up = d.pop("up")
            th4 = d.pop("th4")
            nc.tensor.matmul(
                out=up[:], lhsT=uf8[:, 2:4, :], rhs=th4[:, 2:4, :],
                start=False, stop=True, perf_mode=DR, skip_group_check=True,
            )
            if sh == 0:
                Ebt = ebp.tile([SLAB, HGR], f32, tag="Eb")
                slabst[s] = dict(Eb=Ebt)
                if s + 1 < NSLAB:
                    load_zw_slab(s + 1, zw_bufs[(s + 1) % 2])
            ebt = alp.tile([1, HGR], f32, tag="ebt")
            nc.scalar.activation(out=ebt[:], in_=up[0:1, :], func=AF.Exp)
            nc.sync.dma_start(out=slabst[s]["Eb"][sh : sh + 1, :], in_=ebt[:])

        def softmax_phase(s):
            d = slabst[s]
            Eb = d["Eb"]
            Em = smp.tile([SLAB, HGR], f32, tag="Em")
            nc.vector.tensor_tensor(
                out=Em[:], in0=Eb[:], in1=mask32[:, s, :], op=OP.mult
            )
            S16 = smp.tile([SLAB, 8 + 8], f32, tag="S16")
            nc.vector.tensor_reduce(
                out=S16[:, :8],
                in_=Em[:].rearrange("j (b n) -> j b n", n=64),
                axis=mybir.AxisListType.X, op=OP.add,
            )
            nc.vector.reciprocal(S16[:, 8:], S16[:, :8])
            co = smp.tile([SLAB, HGR], f32, tag="co")
            nc.vector.tensor_tensor(
                out=co[:].rearrange("j (b n) -> j b n", n=64),
                in0=Em[:].rearrange("j (b n) -> j b n", n=64),
                in1=S16[:, 8:].to_broadcast((SLAB, 8, 64)),
                op=OP.mult,
            )
            cob = smp.tile([SLAB, HGR], bf16, tag="cob")
            nc.vector.tensor_tensor(
                out=cob[:], in0=co[:], in1=rwm32[:, s, :], op=OP.add
            )
            cT = ctp.tile([128, HGT, SLAB], bf16, tag="cT")
            for t in range(HGT):
                nc.sync.dma_start_transpose(
                    out=cT[:, t, :], in_=cob[:, 128 * t : 128 * (t + 1)]
                )
            d["cT"] = cT

        def tail_phase(h):
            s = h // SLAB
            sh = h % SLAB
            d = st.pop(h)
            etr4 = d["etr4"]
            cT = slabst[s]["cT"]
            cz = czp.tile([128, HGT, 8], bf16, tag="cz")
            nc.vector.tensor_tensor(
                out=cz[:], in0=czm[:],
                in1=cT[:, :, sh : sh + 1].to_broadcast((128, HGT, 8)),
                op=OP.mult,
            )
            tp = tps.tile([8, DIM], f32, tag="tps")
            for t in range(HGT):
                nc.tensor.matmul(
                    out=tp[:], lhsT=cz[:, t, :], rhs=etr4[:, t, :],
                    start=(t == 0), stop=(t == HGT - 1), skip_group_check=True,
                )
            outS = osp.tile([8, DIM], f32, tag="outS")
            nc.scalar.copy(outS[:], tp[:])
            nc.sync.dma_start(out=out_d[8 * h : 8 * (h + 1), :], in_=outS[:])

        TAIL_LAG = SLAB + 3
        for k in range(NHG + TAIL_LAG + 4):
            if k < NHG:
                proj_phase(k)
            if 1 <= k < NHG + 1:
                proj2_phase(k - 1)
            if 3 <= k < NHG + 3:
                attn_phase(k - 3)
            j = k - 3 - TAIL_LAG
            if 0 <= j < NHG:
                tail_phase(j)
            if 3 <= k < NHG + 3:
                u2_phase(k - 3)
                if (k - 3) % SLAB == SLAB - 1:
                    softmax_phase((k - 3) // SLAB)

    nc.finalize()
    return nc


_NC = None


def _get_nc():
    global _NC
    if _NC is None:
        _NC = build_nc()
    return _NC


def _prep_in_maps(inputs):
    e = np.asarray(inputs["batch_nei_e_emb"], dtype=np.float32).astype(BF)
    rid = np.asarray(inputs["batch_nei_rid"]).astype(np.int32)
    rw = np.asarray(inputs["batch_nei_rw"], dtype=np.float32)
    qr = np.asarray(inputs["batch_q_rid"]).astype(np.int32)

    w = np.asarray(inputs["w_r_weight"], dtype=np.float32)
    nrm = np.maximum(np.linalg.norm(w, axis=1, keepdims=True), 1e-12)
    wn = (w / nrm).astype(BF)  # [N_WR, DIM]
    WT = np.asarray(inputs["attn_W_w"], dtype=np.float32).T  # [in=512, out=512]
    wzf8 = np.ascontiguousarray(
        WT[:256].reshape(2, 128, H).transpose(1, 0, 2)
    ).astype(F8)
    wef8 = np.ascontiguousarray(
        WT[256:].reshape(2, 128, H).transpose(1, 0, 2)
    ).astype(F8)
    zq = np.ascontiguousarray(np.asarray(inputs["zq_weight"], dtype=np.float32)).astype(BF)
    bias = np.asarray(inputs["attn_W_b"], dtype=np.float32).reshape(1, H)
    biasb = np.ascontiguousarray(np.broadcast_to(bias, (128, H)))
    ua = np.asarray(inputs["u_a_w"], dtype=np.float32).reshape(H)
    uf8 = np.ascontiguousarray(np.broadcast_to(ua.reshape(4, 128).T[:, :, None], (128, 4, 128))).astype(F8)

    # onehot for zw-add: oh2[p, i, r] = 1 if r//64 == 4i+p
    rr = np.arange(HGR) // 64
    oh2 = np.zeros((4, 2, HGR), np.float32)
    for p in range(4):
        for i in range(2):
            oh2[p, i] = (rr == 4 * i + p)
    oh2 = oh2.astype(F8)
    # czm[p, t, j] = 1 if j == 2t + p//64
    pp = np.arange(128)[:, None, None] // 64
    tt = np.arange(HGT)[None, :, None]
    jj = np.arange(8)[None, None, :]
    czm = (jj == 2 * tt + pp).astype(BF)

    in_maps = []
    for c in range(NCORES):
        sl = slice(BC * c, BC * (c + 1))
        ec = np.ascontiguousarray(e[sl].reshape(ROWS, DIM))
        ridc = rid[sl].reshape(ROWS)
        rwc = rw[sl].reshape(ROWS)
        qc = qr[sl]
        gc = np.ascontiguousarray(wn[ridc])  # [ROWS, DIM]
        maskc = (ridc < CNT_E).astype(np.float32)
        m32 = np.ascontiguousarray(
            maskc.reshape(NSLAB, SLAB, HGR).transpose(1, 0, 2)
        )
        rwm = np.ascontiguousarray(
            (rwc * maskc).reshape(NSLAB, SLAB, HGR).transpose(1, 0, 2)
        )
        in_maps.append(
            {
                "e": ec,
                "g": gc,
                "wef8": wef8,
                "wzf8": wzf8,
                "uf8": uf8,
                "oh2": oh2,
                "biasb": biasb,
                "mask32": m32,
                "rwm32": rwm,
                "czm": czm,
                "zq": zq,
                "qoff": np.ascontiguousarray(qc.reshape(BC // 128, 128).T),
            }
        )
    return in_maps


def run_cores(inputs, trace=False, tmpdir=None):
    from concourse.bass_utils import run_bass_kernel_spmd

    nc = _get_nc()
    in_maps = _prep_in_maps(inputs)
    res = run_bass_kernel_spmd(
        nc, in_maps, core_ids=list(range(NCORES)), trace=trace, tmpdir=tmpdir
    )
    out = np.concatenate([res.results[c]["out"] for c in range(NCORES)], axis=0)
    return out, res


def kernel(**inputs):
    out, _ = run_cores(inputs, trace=False)
    return out
